# revision 1
# baseline (speedup 1.0000x reference)
"""Multi-head attention (B=2, S=2048, nx=768, H=12) on 8 TRN2 NeuronCores.

Sharding: 24 (batch, head) pairs -> 3 heads per core. Core c handles batch
c//4, heads {3*(c%4), +1, +2}. Each core computes QKV projection for its
head slice, attention, and a partial output projection (its 192 rows of
w_proj); the host sums the 4 partials per batch and adds b_proj.

Device pipeline (per core, matmul operands bf16, accumulation f32):
  - Inputs arrive pre-transposed/sliced from host:
      xt  [896, 2048]  = [hidden[b].T ; ones-row ; zero-pad]  (bias trick)
      wqk [896, 384]   = w_attn cols [qA kA qB kB qC kC] + bias row
      wv  [896, 192]   = w_attn v cols [vA vB vC] + bias row
      wp  [192, 768]   = w_proj rows for the 3 heads
  - QK proj emits Q^T/K^T directly ([d, s] layout) so the transposed-score
    matmul S^T[t, q] = K Q^T needs no transposes anywhere.
  - softmax: scores are small (no max subtraction needed); exp on ACT with
    fused 1/8 scale; denominator via a ones-column appended to V (the PV
    matmul emits sum(exp) as psum row 64); normalize with reciprocal +
    K=1-matmul partition broadcast.
  - Head pipeline: PV for head h runs t-major (4 open psum accumulations,
    one per 512-query chunk) interleaved with scores+exp for head h+1, so
    the scalar engine (exp is its ~110us floor) never starves and the PE
    never idles long enough to re-throttle (HAM).
  - output proj consumes a^T [192, s] as lhsT, wp as rhs -> natural [s, n]
    partial, staged through SBUF to DRAM.
"""

import numpy as np
import ml_dtypes

import concourse.bass as bass
import concourse.tile as tile
import concourse.mybir as mybir
from concourse import bacc

BF16 = mybir.dt.bfloat16
F32 = mybir.dt.float32

NX = 768
D = 64
HPC = 3          # heads per core
N_CORES = 8
KCH = 7          # contraction chunks of 128 (768 data + bias row + pad)
KDIM = KCH * 128  # 896


def build_nc(S=2048):
    """Build the single-core SPMD program. S = sequence length."""
    TC = S // 128    # t (key) chunks
    QC = S // 512    # q chunks of 512
    nc = bacc.Bacc("TRN2", target_bir_lowering=False, debug=False)

    xt_d = nc.dram_tensor("xt", [KDIM, S], BF16, kind="ExternalInput")
    wqk_d = nc.dram_tensor("wqk", [KDIM, 6 * D], BF16, kind="ExternalInput")
    wv_d = nc.dram_tensor("wv", [KDIM, HPC * D], BF16, kind="ExternalInput")
    wp_d = nc.dram_tensor("wp", [HPC * D, NX], BF16, kind="ExternalInput")
    out_d = nc.dram_tensor("out", [S, NX], F32, kind="ExternalOutput")

    with tile.TileContext(nc) as tc:
        _build_body(tc, out_d.ap(), xt_d.ap(), wqk_d.ap(), wv_d.ap(),
                    wp_d.ap(), S, TC, QC)
    nc.compile()
    return nc


RB_PSUM_DIRECT = True  # feed tensor_tensor in1 from PSUM (skip rb copy)


def _build_body(tc, out_d, xt_d, wqk_d, wv_d, wp_d, S, TC, QC):
    nc = tc.nc
    P = 128
    NHALF = S // 1024  # exp calls per t-chunk, each [128, 1024]

    with tc.tile_pool(name="const", bufs=1) as cpool, \
         tc.tile_pool(name="epool", bufs=TC + 2) as epool, \
         tc.tile_pool(name="small", bufs=3) as spool, \
         tc.tile_pool(name="pvpool", bufs=QC + 1) as pvpool, \
         tc.tile_pool(name="ps_score", bufs=2, space="PSUM") as ps_score, \
         tc.tile_pool(name="ps_pv", bufs=QC, space="PSUM") as ps_pv:

        # ---- stage inputs in SBUF ----
        xt_sb = cpool.tile([P, KCH, S], BF16)
        nc.sync.dma_start(xt_sb[:], xt_d.rearrange("(c p) s -> p c s", p=P))
        wqk_sb = cpool.tile([P, KCH, 6 * D], BF16)
        nc.sync.dma_start(wqk_sb[:], wqk_d.rearrange("(c p) m -> p c m", p=P))
        wv_sb = cpool.tile([P, KCH, HPC * D], BF16)
        nc.sync.dma_start(wv_sb[:], wv_d.rearrange("(c p) m -> p c m", p=P))
        wp0_sb = cpool.tile([P, NX], BF16)
        nc.sync.dma_start(wp0_sb[:], wp_d[0:P, :])
        wp1_sb = cpool.tile([D, NX], BF16)
        nc.sync.dma_start(wp1_sb[:], wp_d[P:HPC * D, :])
        ones4 = cpool.tile([97, D], F32)
        nc.vector.memset(ones4[:], 1.0)

        # q2: Q^T duplicated into both partition halves (rows 0:64 == 64:128)
        # k2: K^T with even token-chunks in rows 0:64, odd in rows 64:128 —
        # the stationary layout for the row-paired (tile_position) scores
        # matmuls that run two K=64 contractions concurrently.
        q2_sb = cpool.tile([P, HPC, S], BF16)
        k2_sb = cpool.tile([P, HPC, S // 2], BF16)
        v_sb = cpool.tile([P, TC, HPC, D + 1], BF16)
        aT_ab = cpool.tile([P, S], BF16)   # heads 0,1 stacked
        aT_c = cpool.tile([D, S], BF16)    # head 2

        # wqk col order is [qA kA qB kB qC kC]; m-chunk mc covers head mc's
        # q (psum partitions 0:64) and k (64:128).
        def qk_proj(mc):
            for qc in range(QC):
                ps = ps_score.tile([P, 1024], F32, tag="score")
                for kc in range(KCH):
                    nc.tensor.matmul(
                        ps[:, 0:512],
                        wqk_sb[:, kc, mc * 128:(mc + 1) * 128],
                        xt_sb[:, kc, qc * 512:(qc + 1) * 512],
                        start=(kc == 0), stop=(kc == KCH - 1))
                nc.vector.tensor_copy(q2_sb[0:D, mc, qc * 512:(qc + 1) * 512],
                                      ps[0:D, 0:512])
                kview = ps[D:P, 0:512].rearrange("p (b c) -> p b c", c=128)
                k2w = k2_sb[:, mc, qc * 256:(qc + 1) * 256].rearrange(
                    "p (b c) -> p b c", c=128)
                nc.vector.tensor_copy(k2w[0:D], kview[:, 0::2, :])
                nc.vector.tensor_copy(k2w[D:P], kview[:, 1::2, :])
            # duplicate q into the lower half (bf16 SBUF copy runs in DVE
            # 4x mode, ~0.6us for the whole head)
            nc.vector.tensor_copy(q2_sb[D:P, mc, :], q2_sb[0:D, mc, :])

        def v_proj():
            nc.vector.memset(v_sb[:, :, :, D:D + 1], 1.0)
            for t in range(TC):
                ps = ps_score.tile([P, 1024], F32, tag="score")
                for kc in range(KCH):
                    nc.tensor.matmul(
                        ps[:, 0:HPC * D],
                        xt_sb[:, kc, t * 128:(t + 1) * 128],
                        wv_sb[:, kc, :],
                        start=(kc == 0), stop=(kc == KCH - 1))
                nc.vector.tensor_copy(
                    v_sb[:, t, :, 0:D],
                    ps[:, 0:HPC * D].rearrange("p (h d) -> p h d", h=HPC))

        e_tiles = {}

        def scores_exp_pair(h, j):
            # two t-chunks (2j, 2j+1) computed concurrently as row-tiles of
            # the PE array: top rows contract k2[0:64], bottom k2[64:128].
            eA = epool.tile([P, S], BF16, tag="E", name=f"eA_{h}_{j}")
            eB = epool.tile([P, S], BF16, tag="E", name=f"eB_{h}_{j}")
            e_tiles[(h, 2 * j)] = eA
            e_tiles[(h, 2 * j + 1)] = eB
            for half in range(NHALF):
                psA = ps_score.tile([P, 1024], F32, tag="score", name="psA")
                psB = ps_score.tile([P, 1024], F32, tag="score", name="psB")
                for qq in range(2):
                    qsl = slice((half * 2 + qq) * 512,
                                (half * 2 + qq + 1) * 512)
                    nc.tensor.matmul(
                        psA[:, qq * 512:(qq + 1) * 512],
                        k2_sb[0:D, h, j * 128:(j + 1) * 128],
                        q2_sb[0:D, h, qsl], start=True, stop=True)
                    nc.tensor.matmul(
                        psB[:, qq * 512:(qq + 1) * 512],
                        k2_sb[D:P, h, j * 128:(j + 1) * 128],
                        q2_sb[D:P, h, qsl], start=True, stop=True)
                nc.scalar.activation(
                    eA[:, half * 1024:(half + 1) * 1024], psA[:],
                    mybir.ActivationFunctionType.Exp, scale=0.125)
                nc.scalar.activation(
                    eB[:, half * 1024:(half + 1) * 1024], psB[:],
                    mybir.ActivationFunctionType.Exp, scale=0.125)

        def norm_head(h, pvs):
            # sumexp rows gathered at 32-aligned partitions -> one batched
            # reciprocal per head (the [1,512] reciprocal is single-lane and
            # costs 3.3us; batching 4 rows shares that cost).
            rt = spool.tile([97, 512], F32, tag="rt")
            nc.vector.memset(rt[:], 1.0)
            for qc in range(QC):
                nc.vector.tensor_copy(rt[32 * qc:32 * qc + 1, :],
                                      pvs[qc][D:D + 1, :])
            rr = spool.tile([97, 512], F32, tag="rr")
            nc.vector.reciprocal(rr[:], rt[:])
            pvsbs = []
            for qc in range(QC):
                pvsb = pvpool.tile([D, 512], F32, tag="pvsb",
                                   name=f"pvsb_{h}_{qc}")
                nc.vector.tensor_copy(pvsb[:], pvs[qc][0:D, :])
                pvsbs.append(pvsb)
            for qc in range(QC):
                rb = ps_score.tile([P, 1024], F32, tag="score", name="rb")
                nc.tensor.matmul(rb[0:D, 0:512],
                                 ones4[32 * qc:32 * qc + 1, :],
                                 rr[32 * qc:32 * qc + 1, :],
                                 start=True, stop=True,
                                 tile_position=(32 * qc, 0))
                if RB_PSUM_DIRECT:
                    rb_src = rb[0:D, 0:512]
                else:
                    rb_sb = spool.tile([D, 512], F32, tag="rb_sb",
                                       name=f"rbsb_{h}_{qc}")
                    nc.vector.tensor_copy(rb_sb[:], rb[0:D, 0:512])
                    rb_src = rb_sb[:]
                dst = (aT_ab[h * D:(h + 1) * D, qc * 512:(qc + 1) * 512]
                       if h < 2 else aT_c[:, qc * 512:(qc + 1) * 512])
                nc.vector.tensor_tensor(dst, pvsbs[qc][:], rb_src,
                                        mybir.AluOpType.mult)

        # ---- emission order = pipeline order ----
        qk_proj(0)
        for j in range(TC // 2):
            scores_exp_pair(0, j)
        v_proj()
        qk_proj(1)
        qk_proj(2)

        for h in range(HPC):
            pvs = [ps_pv.tile([P, 512], F32, tag="pv", name=f"pv_{h}_{qc}")
                   for qc in range(QC)]
            for t in range(TC):
                if h + 1 < HPC and t % 2 == 0:
                    scores_exp_pair(h + 1, t // 2)
                e = e_tiles.pop((h, t))
                for qc in range(QC):
                    nc.tensor.matmul(
                        pvs[qc][0:D + 1, :],
                        v_sb[:, t, h, :],
                        e[:, qc * 512:(qc + 1) * 512],
                        start=(t == 0), stop=(t == TC - 1))
            # proj chunk: out[sc*128:(sc+1)*128, :] partial -> DRAM; PSUM->
            # SBUF staging alternates Vector/Scalar so neither engine gates
            # the tail.
            def proj_chunk(sc):
                s_sl = slice(sc * 128, (sc + 1) * 128)
                for n0, nw in ((0, 512), (512, 256)):
                    ps = ps_score.tile([P, 1024], F32, tag="score",
                                       name=f"proj_{sc}_{n0}")
                    nc.tensor.matmul(ps[:, 0:nw], aT_ab[:, s_sl],
                                     wp0_sb[:, n0:n0 + nw],
                                     start=True, stop=False)
                    nc.tensor.matmul(ps[:, 0:nw], aT_c[:, s_sl],
                                     wp1_sb[:, n0:n0 + nw],
                                     start=False, stop=True)
                    ostage = spool.tile([P, 512], F32, tag="ostage",
                                        name=f"ostage_{sc}_{n0}")
                    if n0 == 0:
                        nc.vector.tensor_copy(ostage[:, 0:nw], ps[:, 0:nw])
                    else:
                        nc.scalar.copy(ostage[:, 0:nw], ps[:, 0:nw])
                    nc.sync.dma_start(out_d[s_sl, n0:n0 + nw],
                                      ostage[:, 0:nw])

            norm_head(h, pvs)
            if h == HPC - 1:
                for sc in range(S // 128):
                    proj_chunk(sc)


# ---------------------------------------------------------------------------
# host side
# ---------------------------------------------------------------------------

def make_in_maps(hidden_states, w_attn, b_attn, w_proj, S=2048):
    """Build the 8 per-core input dicts (numpy bf16)."""
    bf = ml_dtypes.bfloat16
    hidden = np.asarray(hidden_states)
    w_attn = np.asarray(w_attn)
    b_attn = np.asarray(b_attn)
    w_proj = np.asarray(w_proj)

    xts = []
    for b in range(hidden.shape[0]):
        xt = np.zeros((KDIM, S), dtype=bf)
        xt[0:NX, :] = hidden[b].T.astype(bf)
        xt[NX, :] = 1.0
        xts.append(xt)

    in_maps = []
    for c in range(N_CORES):
        b = c // (N_CORES // hidden.shape[0])
        h0 = HPC * (c % (N_CORES // hidden.shape[0]))
        wqk = np.zeros((KDIM, 6 * D), dtype=bf)
        wv = np.zeros((KDIM, HPC * D), dtype=bf)
        for i in range(HPC):
            h = h0 + i
            wqk[0:NX, (2 * i) * D:(2 * i + 1) * D] = \
                w_attn[:, h * D:(h + 1) * D].astype(bf)
            wqk[NX, (2 * i) * D:(2 * i + 1) * D] = \
                b_attn[h * D:(h + 1) * D].astype(bf)
            wqk[0:NX, (2 * i + 1) * D:(2 * i + 2) * D] = \
                w_attn[:, NX + h * D:NX + (h + 1) * D].astype(bf)
            wqk[NX, (2 * i + 1) * D:(2 * i + 2) * D] = \
                b_attn[NX + h * D:NX + (h + 1) * D].astype(bf)
            wv[0:NX, i * D:(i + 1) * D] = \
                w_attn[:, 2 * NX + h * D:2 * NX + (h + 1) * D].astype(bf)
            wv[NX, i * D:(i + 1) * D] = \
                b_attn[2 * NX + h * D:2 * NX + (h + 1) * D].astype(bf)
        wp = w_proj[h0 * D:(h0 + HPC) * D, :].astype(bf)
        in_maps.append({"xt": xts[b], "wqk": wqk, "wv": wv, "wp": wp})
    return in_maps


_CACHE = {}


def kernel(hidden_states, w_attn, b_attn, w_proj, b_proj):
    from concourse.bass_utils import run_bass_kernel_spmd

    hidden = np.asarray(hidden_states, dtype=np.float32)
    B, S, _ = hidden.shape
    in_maps = make_in_maps(hidden, w_attn, b_attn, w_proj, S=S)

    if S not in _CACHE:
        _CACHE[S] = build_nc(S=S)
    nc = _CACHE[S]

    res = run_bass_kernel_spmd(nc, in_maps, core_ids=list(range(N_CORES)))
    cpb = N_CORES // B
    out = np.zeros((B, S, NX), dtype=np.float32)
    for c in range(N_CORES):
        out[c // cpb] += res.results[c]["out"]
    out += np.asarray(b_proj, dtype=np.float32)
    return out



# revision 3
# speedup vs baseline: 1.2498x; 1.2498x over previous
"""Multi-head attention (B=2, S=2048, nx=768, H=12) on 8 TRN2 NeuronCores.

Sharding: 24 (batch, head) pairs -> 3 heads per core. Core c handles batch
c//4, heads {3*(c%4), +1, +2}. Each core computes QKV projection for its
head slice, attention, and a partial output projection (its 192 rows of
w_proj); the host sums the 4 partials per batch and adds b_proj.

Device pipeline (per core, matmul operands bf16, accumulation f32), laid
out so the PE issues long same-PSUM-bank accumulation chains (pipelined
~N/2.4 ns/matmul instead of the isolated (398+N)/2.4 rate that PSUM-bank
cycling causes) and the Scalar engine (exp is its ~107us floor) is
saturated from the first score tile to the last:

  - qk proj is contraction(kc)-outer over 4 concurrently-open PSUM tiles,
    consuming each xt DMA chunk as it arrives; the bias is applied by DVE
    (tensor_scalar) during the PSUM->SBUF copy, so the contraction is 6
    chunks (768 rows), not 7.
  - head-0 scores+exp pairs are interleaved with v proj and qk proj of
    heads 1/2 so the PE never idles >3.4us (HAM stays at K=8/8).
  - PV for heads 0/1 runs qc-outer in 4-t-chunk chains (one PSUM bank per
    qc, 16-matmul accumulation groups); scores for head h+1 interleave
    between chains. PV for head 2 runs t-outer, trailing the exp stream,
    so only ~2us of PV remains after the last exp.
  - softmax denominator: ones-column appended to V emits sum(exp) as psum
    row 64; normalization via reciprocal_approx_fast (18 bits, plenty) +
    K=1-matmul partition broadcast.
  - output proj consumes a^T [192, s] as lhsT, wp as rhs -> natural [s, n]
    partial in bf16 (host sums partials in f32).
"""

import numpy as np
import ml_dtypes

import concourse.bass as bass
import concourse.tile as tile
import concourse.mybir as mybir
from concourse import bacc

BF16 = mybir.dt.bfloat16
F32 = mybir.dt.float32

NX = 768
D = 64
HPC = 3          # heads per core
N_CORES = 8
KQ = 6           # contraction chunks (128 rows) for q/k proj (no bias row)
KV = 7           # contraction chunks for v proj (includes bias/ones row)
KDIM = KV * 128  # 896


def build_nc(S=2048):
    """Build the single-core SPMD program. S = sequence length."""
    TC = S // 128    # t (key) chunks
    QC = S // 512    # q chunks of 512
    nc = bacc.Bacc("TRN2", target_bir_lowering=False, debug=False)

    xt_d = nc.dram_tensor("xt", [KDIM, S], BF16, kind="ExternalInput")
    wqk_d = nc.dram_tensor("wqk", [KDIM, 6 * D], BF16, kind="ExternalInput")
    wv_d = nc.dram_tensor("wv", [KDIM, HPC * D], BF16, kind="ExternalInput")
    wp_d = nc.dram_tensor("wp", [HPC * D, NX], BF16, kind="ExternalInput")
    out_d = nc.dram_tensor("out", [S, NX], BF16, kind="ExternalOutput")

    with tile.TileContext(nc) as tc:
        _build_body(tc, out_d.ap(), xt_d.ap(), wqk_d.ap(), wv_d.ap(),
                    wp_d.ap(), S, TC, QC)
    nc.compile()
    return nc


def _build_body(tc, out_d, xt_d, wqk_d, wv_d, wp_d, S, TC, QC):
    nc = tc.nc
    P = 128

    with tc.tile_pool(name="const", bufs=1) as cpool, \
         tc.tile_pool(name="epool", bufs=TC + 8) as epool, \
         tc.tile_pool(name="spool", bufs=2) as spool, \
         tc.tile_pool(name="ps_sc", bufs=2, space="PSUM") as ps_sc, \
         tc.tile_pool(name="ps_sm", bufs=4, space="PSUM") as ps_sm:

        # ---- stage inputs in SBUF (order = arrival order on the queue) ----
        wqk_sb = cpool.tile([P, KQ, 6 * D], BF16)
        nc.sync.dma_start(
            wqk_sb[:], wqk_d[0:KQ * P, :].rearrange("(c p) m -> p c m", p=P))
        # bias row: wqk row 768 is [qb_A(64) kb_A(64) qb_B kb_B qb_C kb_C];
        # flat col h*128+r maps to per-partition scalars (q in rows 0:64,
        # k in rows 64:128).
        bqk_raw = cpool.tile([P, HPC], BF16)
        nc.sync.dma_start(
            bqk_raw[:],
            wqk_d[KQ * P:KQ * P + 1, :].rearrange("o (h p) -> (o p) h", p=P))
        bqk_sb = cpool.tile([P, HPC], F32)
        nc.vector.tensor_copy(bqk_sb[:], bqk_raw[:])
        xt_sb = []
        for kc in range(KQ):
            xt_c = cpool.tile([P, S], BF16, name=f"xt{kc}")
            nc.sync.dma_start(xt_c[:], xt_d[kc * P:(kc + 1) * P, :])
            xt_sb.append(xt_c)
        wv_sb = cpool.tile([P, KV, HPC * D], BF16)
        nc.sync.dma_start(wv_sb[:], wv_d.rearrange("(c p) m -> p c m", p=P))
        xt_c = cpool.tile([P, S], BF16, name="xt6")
        nc.sync.dma_start(xt_c[:], xt_d[KQ * P:KV * P, :])
        xt_sb.append(xt_c)
        wp0_sb = cpool.tile([P, NX], BF16)
        nc.sync.dma_start(wp0_sb[:], wp_d[0:P, :])
        wp1_sb = cpool.tile([D, NX], BF16)
        nc.sync.dma_start(wp1_sb[:], wp_d[P:HPC * D, :])

        ones4 = cpool.tile([97, D], F32)
        nc.vector.memset(ones4[:], 1.0)
        # rt rows 32*qc hold sum(exp); other rows stay 1.0 (only the 32*qc
        # rows are ever consumed, but reciprocal runs on the whole tile).
        rt = cpool.tile([97, 512], F32)
        nc.vector.memset(rt[:], 1.0)

        # q2: Q^T duplicated into both partition halves (rows 0:64 == 64:128)
        # k2: K^T with even token-chunks in rows 0:64, odd in rows 64:128 —
        # the stationary layout for the row-paired (tile_position) scores
        # matmuls that run two K=64 contractions on separate row groups.
        q2_sb = cpool.tile([P, HPC, S], BF16)
        k2_sb = cpool.tile([P, HPC, S // 2], BF16)
        v_sb = cpool.tile([P, TC, HPC, D + 1], BF16)
        aT_ab = cpool.tile([P, S], BF16)   # heads 0,1 stacked
        aT_c = cpool.tile([D, S], BF16)    # head 2
        nc.vector.memset(v_sb[:, :, :, D:D + 1], 1.0)

        # wqk col order is [qA kA qB kB qC kC]; m-chunk mc covers head mc's
        # q (psum partitions 0:64) and k (64:128). kc-outer: 4 open psum
        # accumulations so each xt chunk is consumed as its DMA lands.
        def qk_proj(mc):
            pss = [ps_sm.tile([P, 512], F32, tag="sm", name=f"qk{mc}_{qc}")
                   for qc in range(QC)]
            for kc in range(KQ):
                for qc in range(QC):
                    nc.tensor.matmul(
                        pss[qc][:],
                        wqk_sb[:, kc, mc * 128:(mc + 1) * 128],
                        xt_sb[kc][:, qc * 512:(qc + 1) * 512],
                        start=(kc == 0), stop=(kc == KQ - 1))
            for qc in range(QC):
                ps = pss[qc]
                nc.vector.tensor_scalar_add(
                    q2_sb[0:D, mc, qc * 512:(qc + 1) * 512],
                    ps[0:D, :], bqk_sb[0:D, mc:mc + 1])
                kview = ps[D:P, :].rearrange("p (b c) -> p b c", c=128)
                k2w = k2_sb[:, mc, qc * 256:(qc + 1) * 256].rearrange(
                    "p (b c) -> p b c", c=128)
                nc.vector.tensor_scalar_add(
                    k2w[0:D], kview[:, 0::2, :], bqk_sb[D:P, mc:mc + 1])
                nc.vector.tensor_scalar_add(
                    k2w[D:P], kview[:, 1::2, :], bqk_sb[D:P, mc:mc + 1])
            nc.vector.tensor_copy(q2_sb[D:P, mc, :], q2_sb[0:D, mc, :])

        def v_t(t):
            ps = ps_sm.tile([P, 512], F32, tag="sm", name=f"v_{t}")
            for kc in range(KV):
                nc.tensor.matmul(
                    ps[:, 0:HPC * D],
                    xt_sb[kc][:, t * 128:(t + 1) * 128],
                    wv_sb[:, kc, :],
                    start=(kc == 0), stop=(kc == KV - 1))
            nc.vector.tensor_copy(
                v_sb[:, t, :, 0:D],
                ps[:, 0:HPC * D].rearrange("p (h d) -> p h d", h=HPC))

        e_tiles = {}

        def sc_pair(h, j):
            # two t-chunks (2j, 2j+1) as row-tiles of the PE array: psA
            # contracts k2 rows 0:64 (even chunk), psB rows 64:128 (odd).
            # Each tile's two 512-col matmuls run back-to-back.
            eA = epool.tile([P, S], BF16, tag="E", name=f"eA_{h}_{j}")
            eB = epool.tile([P, S], BF16, tag="E", name=f"eB_{h}_{j}")
            e_tiles[(h, 2 * j)] = eA
            e_tiles[(h, 2 * j + 1)] = eB
            for half in range(2):
                psA = ps_sc.tile([P, 1024], F32, tag="sc", name="psA")
                for qq in range(2):
                    qsl = slice((half * 2 + qq) * 512,
                                (half * 2 + qq + 1) * 512)
                    nc.tensor.matmul(
                        psA[:, qq * 512:(qq + 1) * 512],
                        k2_sb[0:D, h, j * 128:(j + 1) * 128],
                        q2_sb[0:D, h, qsl], start=True, stop=True)
                psB = ps_sc.tile([P, 1024], F32, tag="sc", name="psB")
                for qq in range(2):
                    qsl = slice((half * 2 + qq) * 512,
                                (half * 2 + qq + 1) * 512)
                    nc.tensor.matmul(
                        psB[:, qq * 512:(qq + 1) * 512],
                        k2_sb[D:P, h, j * 128:(j + 1) * 128],
                        q2_sb[D:P, h, qsl], start=True, stop=True)
                nc.scalar.activation(
                    eA[:, half * 1024:(half + 1) * 1024], psA[:],
                    mybir.ActivationFunctionType.Exp, scale=0.125)
                nc.scalar.activation(
                    eB[:, half * 1024:(half + 1) * 1024], psB[:],
                    mybir.ActivationFunctionType.Exp, scale=0.125)

        def pv4(h, qc, g, pvs):
            # 4 consecutive accumulating matmuls into the same psum bank.
            for tt in range(4):
                t = 4 * g + tt
                nc.tensor.matmul(
                    pvs[qc][0:D + 1, :],
                    v_sb[:, t, h, :],
                    e_tiles[(h, t)][:, qc * 512:(qc + 1) * 512],
                    start=(t == 0), stop=(t == TC - 1))

        def norm_head(h, pvs):
            for qc in range(QC):
                nc.vector.tensor_copy(rt[32 * qc:32 * qc + 1, :],
                                      pvs[qc][D:D + 1, :])
            rr = spool.tile([97, 512], F32, tag="rr")
            nc.vector.reciprocal_approx_fast(rr[:], rt[:])
            pvsbs = []
            for qc in range(QC):
                pvsb = spool.tile([D, 512], F32, tag="pvsb", bufs=6,
                                  name=f"pvsb_{h}_{qc}")
                nc.vector.tensor_copy(pvsb[:], pvs[qc][0:D, :])
                pvsbs.append(pvsb)
            for qc in range(QC):
                rb = ps_sm.tile([P, 512], F32, tag="sm", name=f"rb_{h}_{qc}")
                nc.tensor.matmul(rb[0:D, :],
                                 ones4[32 * qc:32 * qc + 1, :],
                                 rr[32 * qc:32 * qc + 1, :],
                                 start=True, stop=True,
                                 tile_position=(32 * qc, 0))
                dst = (aT_ab[h * D:(h + 1) * D, qc * 512:(qc + 1) * 512]
                       if h < 2 else aT_c[:, qc * 512:(qc + 1) * 512])
                nc.vector.tensor_tensor(dst, pvsbs[qc][:], rb[0:D, :],
                                        mybir.AluOpType.mult)

        def proj_chunk(sc_i):
            s_sl = slice(sc_i * 128, (sc_i + 1) * 128)
            psa = ps_sm.tile([P, 512], F32, tag="sm", name=f"pja_{sc_i}")
            nc.tensor.matmul(psa[:], aT_ab[:, s_sl], wp0_sb[:, 0:512],
                             start=True, stop=False)
            nc.tensor.matmul(psa[:], aT_c[:, s_sl], wp1_sb[:, 0:512],
                             start=False, stop=True)
            psb = ps_sm.tile([P, 512], F32, tag="sm", name=f"pjb_{sc_i}")
            nc.tensor.matmul(psb[:, 0:256], aT_ab[:, s_sl],
                             wp0_sb[:, 512:768], start=True, stop=False)
            nc.tensor.matmul(psb[:, 0:256], aT_c[:, s_sl],
                             wp1_sb[:, 512:768], start=False, stop=True)
            ostage = spool.tile([P, NX], BF16, tag="ostage", bufs=3,
                                name=f"ost_{sc_i}")
            nc.vector.tensor_copy(ostage[:, 0:512], psa[:])
            nc.scalar.copy(ostage[:, 512:768], psb[:, 0:256])
            nc.sync.dma_start(out_d[s_sl, :], ostage[:])

        # ---- emission order = pipeline order ----
        qk_proj(0)
        for j in range(TC // 2):
            sc_pair(0, j)
            v_t(2 * j)
            v_t(2 * j + 1)
            if j == 2:
                qk_proj(1)
            if j == 5:
                qk_proj(2)

        for h in range(HPC):
            pvs = [ps_sm.tile([P, 512], F32, tag="sm", name=f"pv_{h}_{qc}")
                   for qc in range(QC)]
            if h + 1 < HPC:
                for g in range(TC // 4):
                    sc_pair(h + 1, 2 * g)
                    pv4(h, 0, g, pvs)
                    pv4(h, 1, g, pvs)
                    sc_pair(h + 1, 2 * g + 1)
                    pv4(h, 2, g, pvs)
                    pv4(h, 3, g, pvs)
            else:
                # trail the exp stream t-outer so only ~2us of PV remains
                # after the last exp; idle slices stay under the HAM window.
                for t in range(TC):
                    e = e_tiles[(h, t)]
                    for qc in range(QC):
                        nc.tensor.matmul(
                            pvs[qc][0:D + 1, :],
                            v_sb[:, t, h, :],
                            e[:, qc * 512:(qc + 1) * 512],
                            start=(t == 0), stop=(t == TC - 1))
            norm_head(h, pvs)
        for sc_i in range(S // 128):
            proj_chunk(sc_i)


# ---------------------------------------------------------------------------
# host side
# ---------------------------------------------------------------------------

def make_in_maps(hidden_states, w_attn, b_attn, w_proj, S=2048):
    """Build the 8 per-core input dicts (numpy bf16)."""
    bf = ml_dtypes.bfloat16
    hidden = np.asarray(hidden_states)
    w_attn = np.asarray(w_attn)
    b_attn = np.asarray(b_attn)
    w_proj = np.asarray(w_proj)

    xts = []
    for b in range(hidden.shape[0]):
        xt = np.zeros((KDIM, S), dtype=bf)
        xt[0:NX, :] = hidden[b].T.astype(bf)
        xt[NX, :] = 1.0
        xts.append(xt)

    in_maps = []
    for c in range(N_CORES):
        b = c // (N_CORES // hidden.shape[0])
        h0 = HPC * (c % (N_CORES // hidden.shape[0]))
        wqk = np.zeros((KDIM, 6 * D), dtype=bf)
        wv = np.zeros((KDIM, HPC * D), dtype=bf)
        for i in range(HPC):
            h = h0 + i
            wqk[0:NX, (2 * i) * D:(2 * i + 1) * D] = \
                w_attn[:, h * D:(h + 1) * D].astype(bf)
            wqk[NX, (2 * i) * D:(2 * i + 1) * D] = \
                b_attn[h * D:(h + 1) * D].astype(bf)
            wqk[0:NX, (2 * i + 1) * D:(2 * i + 2) * D] = \
                w_attn[:, NX + h * D:NX + (h + 1) * D].astype(bf)
            wqk[NX, (2 * i + 1) * D:(2 * i + 2) * D] = \
                b_attn[NX + h * D:NX + (h + 1) * D].astype(bf)
            wv[0:NX, i * D:(i + 1) * D] = \
                w_attn[:, 2 * NX + h * D:2 * NX + (h + 1) * D].astype(bf)
            wv[NX, i * D:(i + 1) * D] = \
                b_attn[2 * NX + h * D:2 * NX + (h + 1) * D].astype(bf)
        wp = w_proj[h0 * D:(h0 + HPC) * D, :].astype(bf)
        in_maps.append({"xt": xts[b], "wqk": wqk, "wv": wv, "wp": wp})
    return in_maps


_CACHE = {}


def kernel(hidden_states, w_attn, b_attn, w_proj, b_proj):
    from concourse.bass_utils import run_bass_kernel_spmd

    hidden = np.asarray(hidden_states, dtype=np.float32)
    B, S, _ = hidden.shape
    in_maps = make_in_maps(hidden, w_attn, b_attn, w_proj, S=S)

    if S not in _CACHE:
        _CACHE[S] = build_nc(S=S)
    nc = _CACHE[S]

    res = run_bass_kernel_spmd(nc, in_maps, core_ids=list(range(N_CORES)))
    cpb = N_CORES // B
    out = np.zeros((B, S, NX), dtype=np.float32)
    for c in range(N_CORES):
        out[c // cpb] += np.asarray(res.results[c]["out"], dtype=np.float32)
    out += np.asarray(b_proj, dtype=np.float32)
    return out


# revision 5
# speedup vs baseline: 1.2516x; 1.0014x over previous
"""Multi-head attention (B=2, S=2048, nx=768, H=12) on 8 TRN2 NeuronCores.

Sharding: 24 (batch, head) pairs -> 3 heads per core. Core c handles batch
c//4, heads {3*(c%4), +1, +2}. Each core computes QKV projection for its
head slice, attention, and a partial output projection (its 192 rows of
w_proj); the host sums the 4 partials per batch and adds b_proj.

Device pipeline (per core, matmul operands bf16, accumulation f32), laid
out so the PE issues long same-PSUM-bank accumulation chains (pipelined
~N/2.4 ns/matmul instead of the isolated (398+N)/2.4 rate that PSUM-bank
cycling causes) and the Scalar engine (exp is its ~107us floor) is
saturated from the first score tile to the last:

  - inputs are host-prepacked partition-major so every DMA is contiguous,
    and are split across the two HWDGE rings (SP + Activation) so the
    ~5.6MB input load halves to ~8us and qk proj starts at ~3us.
  - qk proj is contraction(kc)-outer over 4 concurrently-open PSUM tiles,
    consuming each xt DMA chunk as it arrives; the bias is applied by DVE
    (tensor_scalar) during the PSUM->SBUF copy, so the contraction is 6
    chunks (768 rows), not 7.
  - head-0 scores+exp pairs are interleaved with v proj and qk proj of
    heads 1/2 so the PE never idles >3.4us (HAM stays at K=8/8).
  - scores pairs issue A,B matmuls alternating (distinct 64-row PE tile
    groups) so the two K=64 contractions can overlap when both PSUM
    tiles are free.
  - PV for heads 0/1 runs qc-outer in 8-t-chunk accumulation chains;
    scores for head h+1 interleave between chains. PV for head 2 runs
    t-outer, trailing the exp stream, so only ~2us of PV remains after
    the last exp.
  - softmax denominator: ones-column appended to V emits sum(exp) as psum
    row 64; normalization via reciprocal_approx_fast (18 bits, plenty) +
    K=1-matmul partition broadcast.
  - output proj keeps wp columns stationary (2 LDWEIGHTS per 8 matmuls),
    emitting out^T [n, s] partials in bf16; PSUM->SBUF staging is split
    scalar:vector ~2:1 (scalar reads PSUM at (N+352)/1.2, vector ~3ns/col).
    The host transposes and sums partials in f32.
"""

import numpy as np
import ml_dtypes

import concourse.bass as bass
import concourse.tile as tile
import concourse.mybir as mybir
from concourse import bacc

BF16 = mybir.dt.bfloat16
F32 = mybir.dt.float32

NX = 768
D = 64
HPC = 3          # heads per core
N_CORES = 8
KQ = 6           # contraction chunks (128 rows) for q/k proj (no bias row)
KV = 7           # contraction chunks for v proj (includes bias/ones row)
KDIM = KV * 128  # 896


def build_nc(S=2048):
    """Build the single-core SPMD program. S = sequence length."""
    TC = S // 128    # t (key) chunks
    QC = S // 512    # q chunks of 512
    nc = bacc.Bacc("TRN2", target_bir_lowering=False, debug=False)

    xt_d = nc.dram_tensor("xt", [KDIM, S], BF16, kind="ExternalInput")
    wqk_d = nc.dram_tensor("wqk", [128, KQ * 6 * D], BF16,
                           kind="ExternalInput")
    bqk_d = nc.dram_tensor("bqk", [128, HPC], BF16, kind="ExternalInput")
    wv_d = nc.dram_tensor("wv", [128, KV * HPC * D], BF16,
                          kind="ExternalInput")
    wp_d = nc.dram_tensor("wp", [HPC * D, NX], BF16, kind="ExternalInput")
    out_d = nc.dram_tensor("out", [NX, S], BF16, kind="ExternalOutput")

    with tile.TileContext(nc) as tc:
        _build_body(tc, out_d.ap(), xt_d.ap(), wqk_d.ap(), bqk_d.ap(),
                    wv_d.ap(), wp_d.ap(), S, TC, QC)
    nc.compile()
    return nc


def _build_body(tc, out_d, xt_d, wqk_d, bqk_d, wv_d, wp_d, S, TC, QC):
    nc = tc.nc
    P = 128

    with tc.tile_pool(name="const", bufs=1) as cpool, \
         tc.tile_pool(name="epool", bufs=TC + 10) as epool, \
         tc.tile_pool(name="spool", bufs=2) as spool, \
         tc.tile_pool(name="ps_sc", bufs=2, space="PSUM") as ps_sc, \
         tc.tile_pool(name="ps_sm", bufs=4, space="PSUM") as ps_sm:

        # ---- stage inputs in SBUF, split across the two HWDGE rings ----
        wqk_sb = cpool.tile([P, KQ, 6 * D], BF16)
        nc.sync.dma_start(wqk_sb[:],
                          wqk_d.rearrange("p (c m) -> p c m", c=KQ))
        bqk_raw = cpool.tile([P, HPC], BF16)
        nc.scalar.dma_start(bqk_raw[:], bqk_d[:, :])
        xt_sb = []
        for kc in range(KV):
            xt_c = cpool.tile([P, S], BF16, name=f"xt{kc}")
            eng = nc.sync if kc % 2 == 0 else nc.scalar
            eng.dma_start(xt_c[:], xt_d[kc * P:(kc + 1) * P, :])
            xt_sb.append(xt_c)
        wv_sb = cpool.tile([P, KV, HPC * D], BF16)
        nc.scalar.dma_start(wv_sb[:],
                            wv_d.rearrange("p (c m) -> p c m", c=KV))
        wp0_sb = cpool.tile([P, NX], BF16)
        nc.sync.dma_start(wp0_sb[:], wp_d[0:P, :])
        wp1_sb = cpool.tile([D, NX], BF16)
        nc.sync.dma_start(wp1_sb[:], wp_d[P:HPC * D, :])

        bqk_sb = cpool.tile([P, HPC], F32)
        nc.vector.tensor_copy(bqk_sb[:], bqk_raw[:])
        ones4 = cpool.tile([97, D], F32)
        nc.vector.memset(ones4[:], 1.0)
        # rt rows 32*qc hold sum(exp); other rows stay 1.0 (only the 32*qc
        # rows are ever consumed, but reciprocal runs on the whole tile).
        rt = cpool.tile([97, 512], F32)
        nc.vector.memset(rt[:], 1.0)

        # q2: Q^T duplicated into both partition halves (rows 0:64 == 64:128)
        # k2: K^T with even token-chunks in rows 0:64, odd in rows 64:128 —
        # the stationary layout for the row-paired (tile_position) scores
        # matmuls that run two K=64 contractions on separate row groups.
        q2_sb = cpool.tile([P, HPC, S], BF16)
        k2_sb = cpool.tile([P, HPC, S // 2], BF16)
        v_sb = cpool.tile([P, TC, HPC, D + 1], BF16)
        aT_ab = cpool.tile([P, S], BF16)   # heads 0,1 stacked
        aT_c = cpool.tile([D, S], BF16)    # head 2
        nc.vector.memset(v_sb[:, :, :, D:D + 1], 1.0)

        # wqk col order is [qA kA qB kB qC kC]; m-chunk mc covers head mc's
        # q (psum partitions 0:64) and k (64:128). kc-outer: 4 open psum
        # accumulations so each xt chunk is consumed as its DMA lands.
        def qk_proj(mc):
            pss = [ps_sm.tile([P, 512], F32, tag="sm", name=f"qk{mc}_{qc}")
                   for qc in range(QC)]
            if mc == 0:
                # kc-outer: consume each xt chunk as its DMA lands
                for kc in range(KQ):
                    for qc in range(QC):
                        nc.tensor.matmul(
                            pss[qc][:],
                            wqk_sb[:, kc, mc * 128:(mc + 1) * 128],
                            xt_sb[kc][:, qc * 512:(qc + 1) * 512],
                            start=(kc == 0), stop=(kc == KQ - 1))
            else:
                # inputs resident: same-bank 6-chains
                for qc in range(QC):
                    for kc in range(KQ):
                        nc.tensor.matmul(
                            pss[qc][:],
                            wqk_sb[:, kc, mc * 128:(mc + 1) * 128],
                            xt_sb[kc][:, qc * 512:(qc + 1) * 512],
                            start=(kc == 0), stop=(kc == KQ - 1))
            for qc in range(QC):
                ps = pss[qc]
                nc.vector.tensor_scalar_add(
                    q2_sb[0:D, mc, qc * 512:(qc + 1) * 512],
                    ps[0:D, :], bqk_sb[0:D, mc:mc + 1])
                kview = ps[D:P, :].rearrange("p (b c) -> p b c", c=128)
                k2w = k2_sb[:, mc, qc * 256:(qc + 1) * 256].rearrange(
                    "p (b c) -> p b c", c=128)
                nc.vector.tensor_scalar_add(
                    k2w[0:D], kview[:, 0::2, :], bqk_sb[D:P, mc:mc + 1])
                nc.vector.tensor_scalar_add(
                    k2w[D:P], kview[:, 1::2, :], bqk_sb[D:P, mc:mc + 1])
            nc.vector.tensor_copy(q2_sb[D:P, mc, :], q2_sb[0:D, mc, :])

        def v_t(t):
            ps = ps_sm.tile([P, 512], F32, tag="sm", name=f"v_{t}")
            for kc in range(KV):
                nc.tensor.matmul(
                    ps[:, 0:HPC * D],
                    xt_sb[kc][:, t * 128:(t + 1) * 128],
                    wv_sb[:, kc, :],
                    start=(kc == 0), stop=(kc == KV - 1))
            nc.vector.tensor_copy(
                v_sb[:, t, :, 0:D],
                ps[:, 0:HPC * D].rearrange("p (h d) -> p h d", h=HPC))

        e_tiles = {}

        def sc_pair(h, j):
            # two t-chunks (2j, 2j+1) as row-tiles of the PE array: psA
            # contracts k2 rows 0:64 (even chunk), psB rows 64:128 (odd).
            # A/B alternate so the distinct row groups can overlap.
            eA = epool.tile([P, S], BF16, tag="E", name=f"eA_{h}_{j}")
            eB = epool.tile([P, S], BF16, tag="E", name=f"eB_{h}_{j}")
            e_tiles[(h, 2 * j)] = eA
            e_tiles[(h, 2 * j + 1)] = eB
            for half in range(2):
                psA = ps_sc.tile([P, 1024], F32, tag="sc", name="psA")
                for qq in range(2):
                    qsl = slice((half * 2 + qq) * 512,
                                (half * 2 + qq + 1) * 512)
                    nc.tensor.matmul(
                        psA[:, qq * 512:(qq + 1) * 512],
                        k2_sb[0:D, h, j * 128:(j + 1) * 128],
                        q2_sb[0:D, h, qsl], start=True, stop=True)
                psB = ps_sc.tile([P, 1024], F32, tag="sc", name="psB")
                for qq in range(2):
                    qsl = slice((half * 2 + qq) * 512,
                                (half * 2 + qq + 1) * 512)
                    nc.tensor.matmul(
                        psB[:, qq * 512:(qq + 1) * 512],
                        k2_sb[D:P, h, j * 128:(j + 1) * 128],
                        q2_sb[D:P, h, qsl], start=True, stop=True)
                nc.scalar.activation(
                    eA[:, half * 1024:(half + 1) * 1024], psA[:],
                    mybir.ActivationFunctionType.Exp, scale=0.125)
                nc.scalar.activation(
                    eB[:, half * 1024:(half + 1) * 1024], psB[:],
                    mybir.ActivationFunctionType.Exp, scale=0.125)

        def pv8(h, qc, gg, pvs):
            # 8 consecutive accumulating matmuls into the same psum bank.
            for tt in range(8):
                t = 8 * gg + tt
                nc.tensor.matmul(
                    pvs[qc][0:D + 1, :],
                    v_sb[:, t, h, :],
                    e_tiles[(h, t)][:, qc * 512:(qc + 1) * 512],
                    start=(t == 0), stop=(t == TC - 1))

        def norm_head(h, pvs):
            for qc in range(QC):
                nc.vector.tensor_copy(rt[32 * qc:32 * qc + 1, :],
                                      pvs[qc][D:D + 1, :])
            rr = spool.tile([97, 512], F32, tag="rr")
            nc.vector.reciprocal_approx_fast(rr[:], rt[:])
            pvsbs = []
            for qc in range(QC):
                pvsb = spool.tile([D, 512], F32, tag="pvsb", bufs=6,
                                  name=f"pvsb_{h}_{qc}")
                nc.vector.tensor_copy(pvsb[:], pvs[qc][0:D, :])
                pvsbs.append(pvsb)
            for qc in range(QC):
                rb = ps_sm.tile([P, 512], F32, tag="sm", name=f"rb_{h}_{qc}")
                nc.tensor.matmul(rb[0:D, :],
                                 ones4[32 * qc:32 * qc + 1, :],
                                 rr[32 * qc:32 * qc + 1, :],
                                 start=True, stop=True,
                                 tile_position=(32 * qc, 0))
                dst = (aT_ab[h * D:(h + 1) * D, qc * 512:(qc + 1) * 512]
                       if h < 2 else aT_c[:, qc * 512:(qc + 1) * 512])
                nc.vector.tensor_tensor(dst, pvsbs[qc][:], rb[0:D, :],
                                        mybir.AluOpType.mult)

        def proj_nc(nci):
            # out^T[nci*128:(nci+1)*128, :] — wp columns stationary, aT
            # streams; 2 LDWEIGHTS serve 8 matmuls.
            tiles = [ps_sm.tile([P, 512], F32, tag="sm",
                                name=f"pj_{nci}_{s4}") for s4 in range(QC)]
            nsl = slice(nci * 128, (nci + 1) * 128)
            for s4 in range(QC):
                nc.tensor.matmul(tiles[s4][:], wp0_sb[:, nsl],
                                 aT_ab[:, s4 * 512:(s4 + 1) * 512],
                                 start=True, stop=False)
                nc.tensor.matmul(tiles[s4][:], wp1_sb[:, nsl],
                                 aT_c[:, s4 * 512:(s4 + 1) * 512],
                                 start=False, stop=True)
            for s4 in range(QC):
                ostage = spool.tile([P, 512], BF16, tag="ostage", bufs=4,
                                    name=f"ost_{nci}_{s4}")
                if (nci * QC + s4) % 3 == 2:
                    nc.vector.tensor_copy(ostage[:], tiles[s4][:])
                else:
                    nc.scalar.copy(ostage[:], tiles[s4][:])
                nc.sync.dma_start(out_d[nsl, s4 * 512:(s4 + 1) * 512],
                                  ostage[:])

        # ---- emission order = pipeline order ----
        qk_proj(0)
        for j in range(TC // 2):
            sc_pair(0, j)
            v_t(2 * j)
            v_t(2 * j + 1)
            if j == 2:
                qk_proj(1)
            if j == 5:
                qk_proj(2)

        for h in range(HPC):
            pvs = [ps_sm.tile([P, 512], F32, tag="sm", name=f"pv_{h}_{qc}")
                   for qc in range(QC)]
            if h + 1 < HPC:
                for gg in range(2):
                    for qc in range(QC):
                        pv8(h, qc, gg, pvs)
                        sc_pair(h + 1, 4 * gg + qc)
            else:
                # chains for the first half (E ready), then trail the exp
                # stream t-outer so only ~2us of PV remains after the last
                # exp; idle slices stay under the HAM window.
                for qc in range(QC):
                    pv8(h, qc, 0, pvs)
                for t in range(8, TC):
                    e = e_tiles[(h, t)]
                    for qc in range(QC):
                        nc.tensor.matmul(
                            pvs[qc][0:D + 1, :],
                            v_sb[:, t, h, :],
                            e[:, qc * 512:(qc + 1) * 512],
                            start=(t == 0), stop=(t == TC - 1))
            norm_head(h, pvs)
        for nci in range(NX // 128):
            proj_nc(nci)


# ---------------------------------------------------------------------------
# host side
# ---------------------------------------------------------------------------

def make_in_maps(hidden_states, w_attn, b_attn, w_proj, S=2048):
    """Build the 8 per-core input dicts (numpy bf16)."""
    bf = ml_dtypes.bfloat16
    hidden = np.asarray(hidden_states)
    w_attn = np.asarray(w_attn)
    b_attn = np.asarray(b_attn)
    w_proj = np.asarray(w_proj)

    xts = []
    for b in range(hidden.shape[0]):
        xt = np.zeros((KDIM, S), dtype=bf)
        xt[0:NX, :] = hidden[b].T.astype(bf)
        xt[NX, :] = 1.0
        xts.append(xt)

    in_maps = []
    for c in range(N_CORES):
        b = c // (N_CORES // hidden.shape[0])
        h0 = HPC * (c % (N_CORES // hidden.shape[0]))
        wqk = np.zeros((NX, 6 * D), dtype=np.float32)
        bqk = np.zeros((128, HPC), dtype=np.float32)
        wv = np.zeros((KDIM, HPC * D), dtype=np.float32)
        for i in range(HPC):
            h = h0 + i
            wqk[:, (2 * i) * D:(2 * i + 1) * D] = w_attn[:, h * D:(h + 1) * D]
            wqk[:, (2 * i + 1) * D:(2 * i + 2) * D] = \
                w_attn[:, NX + h * D:NX + (h + 1) * D]
            bqk[0:D, i] = b_attn[h * D:(h + 1) * D]
            bqk[D:128, i] = b_attn[NX + h * D:NX + (h + 1) * D]
            wv[0:NX, i * D:(i + 1) * D] = \
                w_attn[:, 2 * NX + h * D:2 * NX + (h + 1) * D]
            wv[NX, i * D:(i + 1) * D] = \
                b_attn[2 * NX + h * D:2 * NX + (h + 1) * D]
        # partition-major prepack so device DMAs are contiguous
        wqk_r = np.ascontiguousarray(
            wqk.reshape(KQ, 128, 6 * D).transpose(1, 0, 2)
        ).reshape(128, KQ * 6 * D).astype(bf)
        wv_r = np.ascontiguousarray(
            wv.reshape(KV, 128, HPC * D).transpose(1, 0, 2)
        ).reshape(128, KV * HPC * D).astype(bf)
        wp = w_proj[h0 * D:(h0 + HPC) * D, :].astype(bf)
        in_maps.append({"xt": xts[b], "wqk": wqk_r, "bqk": bqk.astype(bf),
                        "wv": wv_r, "wp": wp})
    return in_maps


_CACHE = {}


def kernel(hidden_states, w_attn, b_attn, w_proj, b_proj):
    from concourse.bass_utils import run_bass_kernel_spmd

    hidden = np.asarray(hidden_states, dtype=np.float32)
    B, S, _ = hidden.shape
    in_maps = make_in_maps(hidden, w_attn, b_attn, w_proj, S=S)

    if S not in _CACHE:
        _CACHE[S] = build_nc(S=S)
    nc = _CACHE[S]

    res = run_bass_kernel_spmd(nc, in_maps, core_ids=list(range(N_CORES)))
    cpb = N_CORES // B
    out = np.zeros((B, S, NX), dtype=np.float32)
    for c in range(N_CORES):
        out[c // cpb] += np.asarray(res.results[c]["out"],
                                    dtype=np.float32).T
    out += np.asarray(b_proj, dtype=np.float32)
    return out
